# revision 8
# baseline (speedup 1.0000x reference)
"""CensNet Trainium2 kernel — 8-core SPMD Bass/Tile implementation (v2).

Reference semantics:
  gc1: Xh = relu(P @ (X @ W1) + b1)   with P = sym-normalized (A+I) from edge_index
  Zh = relu(Z)
  3x edge layers (p2/W2, p3/W3, p32/W32):
      sv = Xh @ p.T                      [n]
      m2 = T.T @ (T * sv[:,None])        [m,m]  (symmetric)
      A  = (m2 with diag<-1) * adj_e
      Zh = relu(A/colmax(A) @ (Zh@W) + b)
  gc4: se = Zh @ p4.T; X4 = (T*se) @ (T.T @ (P @ (Xh@W4) + b4g)) + b4
  gc5: softmax(P @ (relu(X4) @ W5) + b5, axis=1)

v2 layout trick: m2 is symmetric, so each core computes the TRANSPOSED slab
  B[j, i] = A[i, j] = m2[j, i] * adj_e[i, j]   (j = own 750 edge rows, i = all 6000)
with the SAME lhsT/rhs GEMM as the row-slab version. Then:
  - colmax(A)[j] = rowmax_i B[j, i]  -> purely local, no AllReduce
  - En partial:  EnT_partial[f, i] = sum_{j own} Y'[j,f] B[j,i]
    with Y' = (Zh_own @ W) / cm[j]  -> only needs OWN Zh rows, no AllGather
  - partials summed with one ReduceScatter per layer ([8,64,750] -> [64,750])
B lives in SBUF as bf16 (no DRAM round trip, full dynamic range — the model's
Zh magnitudes span 1e-4..1e12, so fp8 for A/Y' is not viable).
"""

import sys

for _p in ("/opt/trn_rl_repo", "/root/.axon_site/_ro/trn_rl_repo"):
    if _p not in sys.path:
        sys.path.insert(0, _p)

import numpy as np

import concourse.bass as bass
import concourse.mybir as mybir
import concourse.tile as tile
from concourse import bacc, bass_utils
from concourse.masks import make_identity

F32 = mybir.dt.float32
BF16 = mybir.dt.bfloat16
F8 = mybir.dt.float8e4
U8 = mybir.dt.uint8
PM_DR = mybir.MatmulPerfMode.DoubleRow
AF = mybir.ActivationFunctionType
ALU = mybir.AluOpType

CORES = 8
N, M = 3000, 6000
FV, FE, H1, H2, NCLS = 128, 64, 256, 128, 16
MR = M // CORES   # 750 edge rows per core
MRP = 752         # fp8 LDW needs 4-byte-aligned plane stride
NR = N // CORES   # 375 node rows per core
NPAD = 3072       # n padded to 12 DoubleRow chunks of 256
NDR = 12


def _chunks(total, step):
    return [(s, min(step, total - s)) for s in range(0, total, step)]


NCH = _chunks(N, 128)      # 24 contraction chunks over n
KBL = _chunks(M, 512)      # 12 streaming blocks over i (pass-1 free dim)
IBL = _chunks(MR, 128)     # 6 row blocks within the core's 750 rows (j)
PBL = _chunks(M, 375)      # 16 pass-2 output blocks over i
VBL = _chunks(N, 375)      # 8 vt blocks over n
NLB = _chunks(NR, 128)     # 3 local node blocks
RG = [list(range(CORES))]


def _col_layout(vec, p=128):
    """[L] -> [p, ceil(L/p)] chunk-major (column j holds vec[j*p:(j+1)*p])."""
    L = len(vec)
    ncol = (L + p - 1) // p
    out = np.zeros((p, ncol), np.float32)
    for j in range(ncol):
        seg = vec[j * p:(j + 1) * p]
        out[: len(seg), j] = seg
    return out


def build_program():
    nc = bacc.Bacc("TRN2", target_bir_lowering=False, debug=False,
                   num_devices=CORES)

    dp = lambda name, shape, dt=F32: nc.declare_dram_parameter(name, list(shape), dt, isOutput=False)
    tpad_d = dp("tpad", (NPAD, M), F8)
    tslab_d = dp("tslab", (NPAD, MR), BF16)
    ttslab_d = dp("ttslab", (MR, N), BF16)
    adjt_d = dp("adjt", (MR, M), BF16)
    zslab_d = dp("zslab", (FE, MR))
    ptcol_d = dp("ptcol", (N, NR), BF16)
    ptrow_d = dp("ptrow", (NR, N), BF16)
    xt_d = dp("xt", (FV, N), BF16)
    kiota_d = dp("kiota", (128, 512))
    cidx_d = dp("cidx", (128, 6))
    vfix_d = dp("vfix", (128, 6))
    w1_d = dp("w1", (FV, H1), BF16)
    w2_d = dp("w2", (FE, FE))
    w3_d = dp("w3", (FE, FE))
    w32_d = dp("w32", (FE, FE))
    w4_d = dp("w4", (128, 2, H2))
    w5_d = dp("w5", (H2, NCLS), BF16)
    pv2_d = dp("pv2", (128, 2))
    pv3_d = dp("pv3", (128, 2))
    pv32_d = dp("pv32", (128, 2))
    pv4_d = dp("pv4", (FE, 1))
    b1_d = dp("b1", (128, 2))
    b2_d = dp("b2", (FE, 1))
    b3_d = dp("b3", (FE, 1))
    b32_d = dp("b32", (FE, 1))
    b4g_d = dp("b4g", (1, H2))
    b4_d = dp("b4", (H2, 1))
    b5_d = dp("b5", (NCLS, 1))
    out_d = nc.declare_dram_parameter("out", [NR, NCLS], F32, isOutput=True)

    layers = [("2", w2_d, pv2_d, b2_d), ("3", w3_d, pv3_d, b3_d), ("32", w32_d, pv32_d, b32_d)]

    with tile.TileContext(nc) as tc:
        with (
            tc.tile_pool(name="const", bufs=1) as cst,
            tc.tile_pool(name="zown", bufs=2) as zop,
            tc.tile_pool(name="dram", bufs=1, space="DRAM") as dram,
        ):
            # ------- persistent constants / state -------
            ident = cst.tile([128, 128], F32)
            make_identity(nc, ident[:])
            ones512 = cst.tile([128, 512], F32)
            nc.vector.memset(ones512[:], 1.0)
            kiota = cst.tile([128, 512], F32)
            nc.sync.dma_start(kiota[:], kiota_d[:])
            cidx = cst.tile([128, 6], F32)
            nc.sync.dma_start(cidx[:], cidx_d[:])
            vfix = cst.tile([128, 6], F32)
            nc.sync.dma_start(vfix[:], vfix_d[:])
            w1 = cst.tile([FV, H1], BF16)
            nc.sync.dma_start(w1[:], w1_d[:])
            wl_sb = {}
            pv_sb = {}
            bl_sb = {}
            for nm, wd, pvd, bd in layers:
                wl_sb[nm] = cst.tile([FE, FE], F32, tag=f"w{nm}", name=f"w{nm}sb")
                nc.sync.dma_start(wl_sb[nm][:], wd[:])
                pv_sb[nm] = cst.tile([128, 2], F32, tag=f"pv{nm}", name=f"pv{nm}sb")
                nc.sync.dma_start(pv_sb[nm][:], pvd[:])
                bl_sb[nm] = cst.tile([FE, 1], F32, tag=f"b{nm}", name=f"b{nm}sb")
                nc.sync.dma_start(bl_sb[nm][:], bd[:])
            w4 = cst.tile([128, 2, H2], F32)
            nc.sync.dma_start(w4[:], w4_d[:])
            w5 = cst.tile([H2, NCLS], BF16)
            nc.sync.dma_start(w5[:], w5_d[:])
            pv4 = cst.tile([FE, 1], F32)
            nc.sync.dma_start(pv4[:], pv4_d[:])
            b1 = cst.tile([128, 2], F32)
            nc.sync.dma_start(b1[:], b1_d[:])
            b4g = cst.tile([1, H2], F32)
            nc.sync.dma_start(b4g[:], b4g_d[:])
            b4 = cst.tile([H2, 1], F32)
            nc.sync.dma_start(b4[:], b4_d[:])
            b5 = cst.tile([NCLS, 1], F32)
            nc.sync.dma_start(b5[:], b5_d[:])

            # B slab: bf16 transposed-A rows for this core [j=750 (6x128), i=6016]
            bsb = cst.tile([128, len(IBL), 6016], BF16)

            svcol = {nm: cst.tile([128, len(NCH)], F32, tag=f"sv{nm}", name=f"sv{nm}sb") for nm, *_ in layers}
            for nm, *_ in layers:
                # rows beyond the 56 real nodes of the last chunk are read by
                # the tsv scale; garbage NaN bits there poison the m2 GEMM
                nc.vector.memset(svcol[nm][:], 0.0)
            xh_loc = cst.tile([128, 2, NR], F32)      # core's own XhT columns
            xw4 = cst.tile([128, len(NLB), H2], BF16)  # XW4 for core's node rows
            u_raw = cst.tile([128, len(IBL), H2], BF16)  # T.T @ G (pre-se-scale)
            rm = cst.tile([128, len(IBL)], F32)        # per-layer row max (= colmax)
            inv = cst.tile([128, len(IBL)], F32)       # 1 / rm
            vfixb = cst.tile([128, len(IBL), 512], BF16)
            for ib in range(len(IBL)):
                nc.vector.tensor_scalar(vfixb[:, ib, :], ones512[:],
                                        vfix[:, ib:ib + 1], None, op0=ALU.mult)

            # zown[k] = this core's Zh rows entering layer k, [FE, 750] f32
            zown = [None] * 4
            zown[0] = zop.tile([FE, MR], F32, tag="zown", name="zown0")
            nc.sync.dma_start(zown[0][:], zslab_d[:])
            nc.scalar.activation(zown[0][:], zown[0][:], AF.Relu)

            # DRAM scratch
            sv_gin = dram.tile([3, NR], F32)
            sv_gout = dram.tile([CORES, 3, NR], F32)
            gd_in = dram.tile([N, H2], BF16)
            gd_out = dram.tile([N, H2], BF16)
            rs_in = [dram.tile([CORES, FE, MR], F32, tag=f"rsi{i}", name=f"rsin{i}") for i in range(2)]
            rs_out = [dram.tile([FE, MR], F32, tag=f"rso{i}", name=f"rsout{i}") for i in range(2)]
            vt_in = dram.tile([CORES, H2, 375], F32)
            vt_out = dram.tile([H2, 375], F32)
            ag5_in = dram.tile([NR, NCLS], BF16)
            ag5_out = dram.tile([N, NCLS], BF16)

            # ================= gc1 =================
            with (
                tc.tile_pool(name="g1", bufs=1) as g1,
                tc.tile_pool(name="g1s", bufs=3) as g1s,
                tc.tile_pool(name="g1p", bufs=2, space="PSUM") as g1p,
                tc.tile_pool(name="g1px", bufs=2, space="PSUM") as g1px,
            ):
                xt_sb = g1.tile([FV, N], BF16)
                nc.sync.dma_start(xt_sb[:], xt_d[:])
                xw1 = g1.tile([128, len(NCH), H1], BF16)
                for ci, (ns, nsz) in enumerate(NCH):
                    ps = g1p.tile([128, H1], F32)
                    nc.tensor.matmul(ps[:nsz, :], (xt_sb[:, ns:ns + nsz]), (w1[:]),
                                     start=True, stop=True)
                    nc.scalar.copy(xw1[:nsz, ci, :], ps[:nsz, :])
                psx = [g1px.tile([128, NR], F32, tag=f"psx{hb}", name=f"psx{hb}t") for hb in range(2)]
                for ci, (ns, nsz) in enumerate(NCH):
                    ptc = g1s.tile([128, NR], BF16, tag="ptc")
                    nc.sync.dma_start(ptc[:nsz, :], ptcol_d[ns:ns + nsz, :])
                    for hb in range(2):
                        nc.tensor.matmul(
                            psx[hb][:, :],
                            (xw1[:nsz, ci, hb * 128:(hb + 1) * 128]),
                            (ptc[:nsz, :]),
                            start=(ci == 0), stop=(ci == len(NCH) - 1))
                for hb in range(2):
                    nc.scalar.activation(xh_loc[:, hb, :], psx[hb][:, :], AF.Relu,
                                         bias=b1[:, hb:hb + 1])

            # ============ prep: sv pieces -> tiny AllGather; XW4 local; G + AR ============
            with (
                tc.tile_pool(name="prep", bufs=1) as pr,
                tc.tile_pool(name="prs", bufs=3) as prs,
                tc.tile_pool(name="prp", bufs=2, space="PSUM") as prp,
            ):
                svp = pr.tile([128, 3, len(NLB)], F32)
                for li, (nm, _, _, _) in enumerate(layers):
                    for j, (nl, nlsz) in enumerate(NLB):
                        ps = prp.tile([128, 1], F32, tag="psv")
                        for hb in range(2):
                            nc.tensor.matmul(ps[:nlsz, :], xh_loc[:, hb, nl:nl + nlsz],
                                             pv_sb[nm][:, hb:hb + 1],
                                             start=(hb == 0), stop=(hb == 1))
                        nc.scalar.copy(svp[:nlsz, li, j:j + 1], ps[:nlsz, :])
                        nc.sync.dma_start(sv_gin[li, nl:nl + nlsz],
                                          svp[:nlsz, li, j:j + 1])
                nc.gpsimd.collective_compute(
                    "AllGather", ALU.bypass, replica_groups=RG,
                    ins=[sv_gin[:].opt()], outs=[sv_gout[:].opt()])
                for li, (nm, _, _, _) in enumerate(layers):
                    for ci, (ns, nsz) in enumerate(NCH):
                        lo = ns
                        while lo < ns + nsz:
                            r = lo // NR
                            take = min((r + 1) * NR, ns + nsz) - lo
                            nc.sync.dma_start(
                                svcol[nm][lo - ns:lo - ns + take, ci:ci + 1],
                                sv_gout[r, li, lo - r * NR:lo - r * NR + take]
                                .unsqueeze(-1))
                            lo += take
                for j, (nl, nlsz) in enumerate(NLB):
                    ps = prp.tile([128, H2], F32, tag="psw4")
                    for hb in range(2):
                        nc.tensor.matmul(ps[:nlsz, :], xh_loc[:, hb, nl:nl + nlsz],
                                         w4[:, hb, :], start=(hb == 0), stop=(hb == 1))
                    nc.scalar.copy(xw4[:nlsz, j, :], ps[:nlsz, :])
                # G partial = P[:, own] @ XW4_own, AllReduce to full G [3000, 128]
                for ci, (ns, nsz) in enumerate(NCH):
                    ps = prp.tile([128, H2], F32, tag="psw4")
                    for j, (nl, nlsz) in enumerate(NLB):
                        ptr = prs.tile([128, 128], BF16, tag="ptr")
                        nc.sync.dma_start(ptr[:nlsz, :nsz], ptrow_d[nl:nl + nlsz, ns:ns + nsz])
                        nc.tensor.matmul(ps[:nsz, :], ptr[:nlsz, :nsz], xw4[:nlsz, j, :],
                                         start=(j == 0), stop=(j == len(NLB) - 1))
                    gst = prs.tile([128, H2], BF16, tag="gst")
                    nc.scalar.copy(gst[:nsz, :], ps[:nsz, :])
                    nc.sync.dma_start(gd_in[ns:ns + nsz, :], gst[:nsz, :])
                nc.gpsimd.collective_compute(
                    "AllReduce", ALU.add, replica_groups=RG,
                    ins=[gd_in[:].opt()], outs=[gd_out[:].opt()])

            # ================= edge layers =================
            with (
                tc.tile_pool(name="ts", bufs=16) as tsp,
                tc.tile_pool(name="tsl", bufs=4) as tslp,
                tc.tile_pool(name="adj", bufs=6) as adjp,
                tc.tile_pool(name="mk", bufs=4) as mkp,
                tc.tile_pool(name="tsv", bufs=2) as tsvp,
                tc.tile_pool(name="aux", bufs=4) as auxp,
                tc.tile_pool(name="yp", bufs=2) as ypp,
                tc.tile_pool(name="gb", bufs=1) as gbp,
                tc.tile_pool(name="pm", bufs=3, space="PSUM") as pmp,
                tc.tile_pool(name="p2", bufs=3, space="PSUM") as p2p,
                tc.tile_pool(name="sp", bufs=2, space="PSUM") as spp,
            ):
                tsv_t = {}

                def emit_tsv(li):
                    nm = layers[li][0]
                    t = tsvp.tile([128, NDR, 2, MRP], F8, tag="tsv", name=f"tsv{li}")
                    tsv_t[li] = t
                    for ci in range(2 * NDR):
                        tsl = tslp.tile([128, MRP], BF16, tag="tsl",
                                        name=f"tsl{li}_{ci}")
                        nc.sync.dma_start(tsl[:, :MR], tslab_d[ci * 128:(ci + 1) * 128, :])
                        nc.scalar.activation(t[:, ci // 2, ci % 2, :MR],
                                             tsl[:, :MR], AF.Copy,
                                             scale=svcol[nm][:, ci:ci + 1])
                        nc.vector.memset(t[:, ci // 2, ci % 2, MR:], 0.0)

                def emit_g_and_u():
                    ones1 = gbp.tile([1, 128], F32)
                    nc.vector.memset(ones1[:], 1.0)
                    psb = spp.tile([128, 3, H2], F32, tag="sp", name="psb4g")
                    nc.tensor.matmul(psb[:, 0, :], ones1[:, :], b4g[:, :],
                                     start=True, stop=True)
                    b4gb = gbp.tile([128, H2], F32)
                    nc.scalar.copy(b4gb[:], psb[:, 0, :])
                    g_sb = gbp.tile([128, len(NCH), H2], BF16)
                    for ci, (ns, nsz) in enumerate(NCH):
                        gch = auxp.tile([128, H2], BF16, tag="gch", name=f"gch{ci}")
                        nc.gpsimd.dma_start(gch[:nsz, :], gd_out[ns:ns + nsz, :])
                        nc.vector.tensor_add(g_sb[:nsz, ci, :], gch[:nsz, :],
                                             b4gb[:nsz, :])
                    ups = [spp.tile([128, 3, H2], F32, tag="sp", name=f"ups{t_}")
                           for t_ in range(2)]
                    for ci, (ns, nsz) in enumerate(NCH):
                        tsl = tslp.tile([128, MRP], BF16, tag="tsl",
                                        name=f"tslu{ci}")
                        nc.sync.dma_start(tsl[:, :MR], tslab_d[ci * 128:(ci + 1) * 128, :])
                        for kb, (k0, ksz) in enumerate(IBL):
                            nc.tensor.matmul(ups[kb // 3][:ksz, kb % 3, :],
                                             tsl[:nsz, k0:k0 + ksz],
                                             g_sb[:nsz, ci, :],
                                             start=(ci == 0), stop=(ci == len(NCH) - 1))
                    for kb, (k0, ksz) in enumerate(IBL):
                        nc.scalar.copy(u_raw[:ksz, kb, :], ups[kb // 3][:ksz, kb % 3, :])

                def emit_pass1_kb(li, kb):
                    tsv = tsv_t[li]
                    k0, kbs = KBL[kb]
                    ts_tiles = []
                    for dci in range(NDR):
                        tst = tsp.tile([128, 2, 512], F8, tag="ts", name=f"ts{li}_{kb}_{dci}")
                        nc.sync.dma_start(
                            tst[:, :, :kbs],
                            tpad_d[dci * 256:(dci + 1) * 256, k0:k0 + kbs]
                            .rearrange("(two p) k -> p two k", p=128))
                        ts_tiles.append(tst)
                    for ib, (i0, ibs) in enumerate(IBL):
                        ibp = (ibs + 3) // 4 * 4
                        pm = pmp.tile([128, 512], F32, tag="pm", name=f"pm{li}_{kb}_{ib}")
                        for dci in range(NDR):
                            nc.tensor.matmul(
                                pm[:ibp, :kbs], tsv[:, dci, :, i0:i0 + ibp],
                                ts_tiles[dci][:, :, :kbs],
                                start=(dci == 0), stop=(dci == NDR - 1),
                                perf_mode=PM_DR)
                        adj = adjp.tile([128, 512], BF16, tag="adj", name=f"adj{li}_{kb}_{ib}")
                        nc.scalar.dma_start(adj[:ibs, :kbs],
                                            adjt_d[i0:i0 + ibs, k0:k0 + kbs])
                        bslice = bsb[:ibs, ib, k0:k0 + kbs]
                        nc.vector.tensor_mul(bslice, pm[:ibs, :kbs], adj[:ibs, :kbs])
                        ck = mkp.tile([128, 1], F32, tag="ck", name=f"ck{li}_{kb}_{ib}")
                        nc.vector.tensor_scalar(ck[:ibs, :], cidx[:ibs, ib:ib + 1],
                                                float(-k0), None, op0=ALU.add)
                        mk = mkp.tile([128, 512], U8, tag="mk", name=f"mk{li}_{kb}_{ib}")
                        nc.vector.tensor_scalar(mk[:ibs, :kbs], kiota[:ibs, :kbs],
                                                ck[:ibs, :], None, op0=ALU.is_equal)
                        nc.vector.copy_predicated(bslice, mk[:ibs, :kbs],
                                                  vfixb[:ibs, ib, :kbs])
                        red = mkp.tile([128, 1], F32, tag="red", name=f"red{li}_{kb}_{ib}")
                        nc.vector.reduce_max(red[:ibs, :], bslice,
                                             axis=mybir.AxisListType.X)
                        if kb == 0:
                            nc.vector.tensor_scalar_max(rm[:ibs, ib:ib + 1],
                                                        red[:ibs, :], -3.0e38)
                        else:
                            nc.vector.tensor_max(rm[:ibs, ib:ib + 1],
                                                 rm[:ibs, ib:ib + 1], red[:ibs, :])

                def emit_y(li):
                    nm = layers[li][0]
                    nc.vector.reciprocal(inv[:], rm[:])
                    yp = ypp.tile([128, len(IBL), FE], BF16, tag="yp", name=f"yp{li}")
                    for jc, (j0, jsz) in enumerate(IBL):
                        ps = spp.tile([128, 3, H2], F32, tag="sp", name=f"psy{li}_{jc}")
                        nc.tensor.matmul(ps[:jsz, 0, :FE], zown[li][:, j0:j0 + jsz],
                                         wl_sb[nm][:], start=True, stop=True)
                        nc.scalar.activation(yp[:jsz, jc, :], ps[:jsz, 0, :FE],
                                             AF.Copy, scale=inv[:jsz, jc:jc + 1])
                    return yp

                def emit_pass2(li, yp):
                    for pb, (i0, isz) in enumerate(PBL):
                        ps2 = p2p.tile([FE, 375], F32, tag="p2", name=f"p2_{li}_{pb}")
                        for jc, (j0, jsz) in enumerate(IBL):
                            nc.tensor.matmul(ps2[:, :isz], yp[:jsz, jc, :],
                                             bsb[:jsz, jc, i0:i0 + isz],
                                             start=(jc == 0), stop=(jc == len(IBL) - 1))
                        en = auxp.tile([FE, 375], F32, tag="en", name=f"en{li}_{pb}")
                        nc.scalar.copy(en[:, :isz], ps2[:, :isz])
                        nc.gpsimd.dma_start(
                            rs_in[li % 2][pb // 2, :, (pb % 2) * 375:(pb % 2) * 375 + isz],
                            en[:, :isz])
                    nc.gpsimd.collective_compute(
                        "ReduceScatter", ALU.add, replica_groups=RG,
                        ins=[rs_in[li % 2][:].opt()], outs=[rs_out[li % 2][:].opt()])

                def emit_zown(k):
                    # Zh entering layer k = relu(En_{k-1} + b_{k-1})
                    nm = layers[k - 1][0]
                    z = zop.tile([FE, MR], F32, tag="zown", name=f"zown{k}")
                    zown[k] = z
                    nc.gpsimd.dma_start(z[:], rs_out[(k - 1) % 2][:])
                    nc.scalar.activation(z[:], z[:], AF.Relu, bias=bl_sb[nm][:])

                with nc.named_scope("tsv0"):
                    emit_tsv(0)
                for li in range(3):
                    with nc.named_scope(f"L{li}p1"):
                        for kb in range(len(KBL)):
                            emit_pass1_kb(li, kb)
                            if li == 0 and kb == 5:
                                with nc.named_scope("gu"):
                                    emit_g_and_u()
                            if li < 2 and kb == 7:
                                with nc.named_scope(f"tsv{li + 1}"):
                                    emit_tsv(li + 1)
                    with nc.named_scope(f"L{li}tail"):
                        if li > 0:
                            emit_zown(li)
                        yp = emit_y(li)
                        emit_pass2(li, yp)

            # ================= gc4 =================
            with (
                tc.tile_pool(name="g4", bufs=1) as g4,
                tc.tile_pool(name="g4s", bufs=3) as g4s,
                tc.tile_pool(name="g4p", bufs=2, space="PSUM") as g4p,
                tc.tile_pool(name="g4p1", bufs=1, space="PSUM") as g4p1,
            ):
                with nc.named_scope("gc4"):
                    emit_zown(3)
                    secol = g4.tile([128, len(IBL)], F32)
                    for kb, (k0, ksz) in enumerate(IBL):
                        ps = g4p1.tile([128, 1], F32, tag="small")
                        nc.tensor.matmul(ps[:ksz, :], zown[3][:, k0:k0 + ksz], pv4[:],
                                         start=True, stop=True)
                        nc.scalar.copy(secol[:ksz, kb:kb + 1], ps[:ksz, :])
                    u_sb = g4.tile([128, len(IBL), H2], BF16)
                    for kb, (k0, ksz) in enumerate(IBL):
                        nc.vector.tensor_scalar_mul(u_sb[:ksz, kb, :], u_raw[:ksz, kb, :],
                                                    secol[:ksz, kb:kb + 1])
                    # VT partial = (U*se).T-accum over local edge rows x TT slab; RS over n
                    for vb, (v0, vsz) in enumerate(VBL):
                        ps = g4p.tile([H2, 375], F32, tag="psvt")
                        for kb, (k0, ksz) in enumerate(IBL):
                            ttt = g4s.tile([128, 375], BF16, tag="ttt")
                            nc.sync.dma_start(ttt[:ksz, :vsz], ttslab_d[k0:k0 + ksz, v0:v0 + vsz])
                            nc.tensor.matmul(ps[:, :vsz], (u_sb[:ksz, kb, :]),
                                             (ttt[:ksz, :vsz]),
                                             start=(kb == 0), stop=(kb == len(IBL) - 1))
                        vst = g4s.tile([H2, 375], F32, tag="vst")
                        nc.scalar.copy(vst[:, :vsz], ps[:, :vsz])
                        nc.gpsimd.dma_start(vt_in[vb, :, :vsz], vst[:, :vsz])
                    nc.gpsimd.collective_compute(
                        "ReduceScatter", ALU.add, replica_groups=RG,
                        ins=[vt_in[:].opt()], outs=[vt_out[:].opt()])

            # ================= gc5 + softmax =================
            with (
                tc.tile_pool(name="g5", bufs=1) as g5,
                tc.tile_pool(name="g5s", bufs=3) as g5s,
                tc.tile_pool(name="g5p", bufs=2, space="PSUM") as g5p,
                tc.tile_pool(name="g5pt", bufs=1, space="PSUM") as g5pt,
            ):
                with nc.named_scope("gc5"):
                    xh5 = g5.tile([H2, NR], BF16)
                    nc.gpsimd.dma_start(xh5[:], vt_out[:])
                    nc.scalar.activation(xh5[:], xh5[:], AF.Relu, bias=b4[:])
                    for j, (t0, tsz) in enumerate(NLB):
                        ps = g5p.tile([128, NCLS], F32, tag="psw5")
                        nc.tensor.matmul(ps[:tsz, :], xh5[:, t0:t0 + tsz], w5[:],
                                         start=True, stop=True)
                        x5st = g5s.tile([128, NCLS], BF16, tag="x5st")
                        nc.scalar.copy(x5st[:tsz, :], ps[:tsz, :])
                        nc.sync.dma_start(ag5_in[t0:t0 + tsz, :], x5st[:tsz, :])
                    nc.gpsimd.collective_compute(
                        "AllGather", ALU.bypass, replica_groups=RG,
                        ins=[ag5_in[:].opt()], outs=[ag5_out[:].opt()])
                    xw5a = g5.tile([128, len(NCH), NCLS], BF16)
                    for ci, (ns, nsz) in enumerate(NCH):
                        nc.gpsimd.dma_start(xw5a[:nsz, ci, :], ag5_out[ns:ns + nsz, :])
                    pst = g5pt.tile([NCLS, 375], F32)
                    for ci, (ns, nsz) in enumerate(NCH):
                        ptc = g5s.tile([128, NR], BF16, tag="ptc5")
                        nc.sync.dma_start(ptc[:nsz, :], ptcol_d[ns:ns + nsz, :])
                        nc.tensor.matmul(pst[:, :], (xw5a[:nsz, ci, :]), (ptc[:nsz, :]),
                                         start=(ci == 0), stop=(ci == len(NCH) - 1))
                    st_sb = g5.tile([NCLS, NR], F32)
                    nc.vector.tensor_scalar_add(st_sb[:], pst[:, :], b5[:])
                    outt = g5.tile([128, len(NLB), NCLS], F32)
                    ptp = g5pt.tile([128, len(NLB), NCLS], F32)
                    for j, (t0, tsz) in enumerate(NLB):
                        nc.tensor.transpose(ptp[:tsz, j, :], st_sb[:, t0:t0 + tsz],
                                            ident[:NCLS, :NCLS])
                        red = g5s.tile([128, 1], F32, tag="red5")
                        nc.vector.reduce_max(red[:tsz, :], ptp[:tsz, j, :],
                                             axis=mybir.AxisListType.X)
                        nc.vector.tensor_scalar_mul(red[:tsz, :], red[:tsz, :], -1.0)
                        nc.scalar.activation(outt[:tsz, j, :], ptp[:tsz, j, :], AF.Exp,
                                             bias=red[:tsz, :])
                        ssum = g5s.tile([128, 1], F32, tag="ssum5")
                        nc.vector.reduce_sum(ssum[:tsz, :], outt[:tsz, j, :],
                                             axis=mybir.AxisListType.X)
                        nc.vector.reciprocal(ssum[:tsz, :], ssum[:tsz, :])
                        nc.vector.tensor_scalar_mul(outt[:tsz, j, :], outt[:tsz, j, :],
                                                    ssum[:tsz, :])
                        nc.sync.dma_start(out_d[t0:t0 + tsz, :], outt[:tsz, j, :])

    nc.finalize()
    return nc


def prepare_inputs(inputs):
    f = lambda x: np.ascontiguousarray(np.asarray(x), dtype=np.float32)
    X, Z, adj_e, T = f(inputs["X"]), f(inputs["Z"]), f(inputs["adj_e"]), f(inputs["T"])
    ei = np.asarray(inputs["edge_index"])
    W1, b1 = f(inputs["W1"]), f(inputs["b1"])
    p2, W2, b2 = f(inputs["p2"]), f(inputs["W2"]), f(inputs["b2"])
    p3, W3, b3 = f(inputs["p3"]), f(inputs["W3"]), f(inputs["b3"])
    p32, W32, b32 = f(inputs["p32"]), f(inputs["W32"]), f(inputs["b32"])
    p4, W4 = f(inputs["p4"]), f(inputs["W4"])
    b4g, b4, W5, b5 = f(inputs["b4g"]), f(inputs["b4"]), f(inputs["W5"]), f(inputs["b5"])

    # dense PT = P.T where P is the symmetric-normalized (A+I) propagation matrix
    src = ei[0].astype(np.int64)
    dst = ei[1].astype(np.int64)
    loop = np.arange(N, dtype=np.int64)
    s = np.concatenate([src, loop])
    d = np.concatenate([dst, loop])
    deg = np.zeros(N, np.float32)
    np.add.at(deg, d, np.float32(1.0))
    dinv = np.where(deg > 0, 1.0 / np.sqrt(deg), 0.0).astype(np.float32)
    norm = dinv[s] * dinv[d]
    PT = np.zeros((N, N), np.float32)
    np.add.at(PT, (s, d), norm)

    import ml_dtypes
    bf16 = ml_dtypes.bfloat16
    fp8 = ml_dtypes.float8_e4m3
    Tpad = np.zeros((NPAD, M), np.float32)
    Tpad[:N] = T
    kiota = np.tile(np.arange(512, dtype=np.float32), (128, 1))
    base = dict(
        tpad=np.ascontiguousarray(Tpad.astype(fp8)),
        xt=np.ascontiguousarray(X.T.astype(bf16)),
        kiota=kiota, w1=np.ascontiguousarray(W1.astype(bf16)), w2=W2, w3=W3, w32=W32,
        w4=np.ascontiguousarray(np.transpose(W4.reshape(2, 128, H2), (1, 0, 2))),
        w5=np.ascontiguousarray(W5.astype(bf16)),
        pv2=np.ascontiguousarray(p2[0].reshape(2, 128).T),
        pv3=np.ascontiguousarray(p3[0].reshape(2, 128).T),
        pv32=np.ascontiguousarray(p32[0].reshape(2, 128).T),
        pv4=np.ascontiguousarray(p4[0][:, None]),
        b1=np.ascontiguousarray(b1.reshape(2, 128).T),
        b2=b2[:, None], b3=b3[:, None], b32=b32[:, None],
        b4g=b4g[None, :], b4=b4[:, None], b5=b5[:, None],
    )
    in_maps = []
    for c in range(CORES):
        e0, n0 = c * MR, c * NR
        m = dict(base)
        m["tslab"] = np.ascontiguousarray(Tpad[:, e0:e0 + MR].astype(bf16))
        m["ttslab"] = np.ascontiguousarray(T[:, e0:e0 + MR].T.astype(bf16))
        m["adjt"] = np.ascontiguousarray(adj_e[:, e0:e0 + MR].T.astype(bf16))
        m["zslab"] = np.ascontiguousarray(Z[e0:e0 + MR].T)
        m["ptcol"] = np.ascontiguousarray(PT[:, n0:n0 + NR].astype(bf16))
        m["ptrow"] = np.ascontiguousarray(PT[n0:n0 + NR, :].astype(bf16))
        m["cidx"] = _col_layout(np.arange(e0, e0 + MR, dtype=np.float32), 128)
        m["cidx"][110:, 5] = -1.0  # pad slots beyond row 750 must never match
        m["vfix"] = _col_layout(np.diagonal(adj_e[e0:e0 + MR, e0:e0 + MR]).astype(np.float32), 128)
        in_maps.append({k: (np.ascontiguousarray(v) if v.dtype in (bf16, fp8)
                            else np.ascontiguousarray(v, dtype=np.float32))
                        for k, v in m.items()})
    return in_maps


_CACHE = {}
TRACE = False
LAST_RESULT = None


def kernel(**inputs):
    global LAST_RESULT
    in_maps = prepare_inputs(inputs)
    if "nc" not in _CACHE:
        _CACHE["nc"] = build_program()
    kw = {}
    if TRACE:
        kw = dict(trace=True, trace_cores=[0])
    res = bass_utils.run_bass_kernel_spmd(_CACHE["nc"], in_maps, list(range(CORES)), **kw)
    LAST_RESULT = res
    out = np.concatenate([res.results[c]["out"] for c in range(CORES)], axis=0)
    return out.astype(np.float32)


if __name__ == "__main__":
    import reference
    ins = reference.setup_inputs()
    ins = {k: np.asarray(v) for k, v in ins.items()}
    got = kernel(**ins)
    print("kernel output", got.shape, got.dtype)
